# revision 11
# baseline (speedup 1.0000x reference)
"""Multi-head attention + out-proj + residual + LayerNorm on 8 trn2 cores.

Sharding: (batch, seq-half) -> 8 shards, collective-free. Each core gets
transposed activations (host-prepped) plus shared (transposed) weights and
computes its full [1024, 1024] output block.

v2 design (vs baseline): everything stays in SBUF (no DRAM staging), the
softmax normalize chain runs entirely off the TensorE queue
(recip_approx_fast on DVE + partition_broadcast on gpsimd + one DVE mul),
and K/Q projection matmul groups are pumped into the attention sk-loop so
the PE always has independent work while ACT chews the exp stream. A
fraction of the exp work can be offloaded to gpsimd via (1+x/16)^16.

  phase V: V_all[Sk, H, dv] in SBUF (+ones col per head)
  loop c (head pair): K-proj/Q-proj for c+1 pumped into attention of c
  attn  : per head pair: scoresT[Sk,Sq] row-tiled pair -> exp (ACT/gpsimd)
          OT[dv+1, Sq] += [V_h|1].T @ expT  (row 64 = softmax denom)
          epilogue: recip_fast -> gpsimd bcast -> DVE mul -> SBUF dma
  final : out = LN(concatT.T @ WpT + q_res) * scale + offset
"""

import os
from contextlib import ExitStack

import numpy as np

import concourse.bass as bass
import concourse.tile as tile
from concourse import bacc, mybir
from concourse._compat import with_exitstack
from concourse.bass_utils import run_bass_kernel_spmd

B, S, D = 4, 2048, 1024
H, DK, DV = 16, 64, 64
F = H * DV            # 1024 flattened head dim (== H*DK)
N_CORES = 8
SQ = S // 2           # 1024 queries per core
SK = S                # 2048 keys per core
P = 128
KD = D // P           # 8 contraction chunks over d_model
NF = F // P           # 8 head-pair chunks
NSK = SK // P         # 16 key chunks
TEMP = float(np.sqrt(D))
EPS = 1e-9

F32 = mybir.dt.float32
BF16 = mybir.dt.bfloat16

# sk indices whose exp runs on gpsimd via (1+x/16)^16 instead of ACT
GP_SKS = ()
# sk indices at which one pumped proj group is emitted
PUMP_SKS = (0, 6, 12)
PUMP_SKS_LAST = (0, 4, 8, 12)

LAST_RESULT = None    # BassKernelResults of the most recent kernel() call


@with_exitstack
def _mha_kernel(ctx: ExitStack, tc: tile.TileContext, out_ap, ins):
    nc = tc.nc
    AF = mybir.ActivationFunctionType
    ALU = mybir.AluOpType

    xq_r = ins["qT"].rearrange("(c p) s -> p c s", p=P)
    xk_r = ins["kT"].rearrange("(c p) s -> p c s", p=P)
    xv_r = ins["vT"].rearrange("(c p) s -> p c s", p=P)

    resident = ctx.enter_context(tc.tile_pool(name="resident", bufs=1))
    # V_all with a ones column appended per head: [sk_part, sk, head, 65]
    v_sb = resident.tile([P, NSK, H, 65], BF16)
    nc.vector.memset(v_sb[:, :, :, 0:1], 1.0)
    # concat.T output of attention: partition = f%128, [128, chunk, q]
    ot_sb = resident.tile([P, NF, SQ], BF16)

    with (
        tc.tile_pool(name="scps", bufs=2, space="PSUM") as scps,
        tc.tile_pool(name="otps", bufs=2, space="PSUM") as otps,
        tc.tile_pool(name="ktp", bufs=2) as ktp,
        tc.tile_pool(name="qtp", bufs=2) as qtp,
        tc.tile_pool(name="expp", bufs=2) as expp,
        tc.tile_pool(name="rcp", bufs=2) as rcp,
        tc.tile_pool(name="bcp", bufs=2) as bcp,
        tc.tile_pool(name="oop", bufs=2) as oop,
    ):
        kt_tiles = {}
        qt_tiles = {}

        def attn(c, sq, work, pump_sks=PUMP_SKS):
            """Attention for head-pair chunk c, query half sq (512 q)."""
            ktc = kt_tiles[c]
            qtc = qt_tiles[c]
            ot_ps = [otps.tile([65, 512], F32, tag="ot", name="otp")
                     for _ in range(2)]

            def emit_scores(sk):
                sc = scps.tile([P, 2, 512], F32, tag="sc", name="sc")
                for hh in range(2):
                    base = hh * 64
                    nc.tensor.matmul(
                        sc[:, hh, :],
                        lhsT=ktc[base:base + 64, sk * P:(sk + 1) * P],
                        rhs=qtc[base:base + 64, sq * 512:(sq + 1) * 512],
                        start=True,
                        stop=True,
                    )
                return sc

            sc_prev = emit_scores(0)
            for sk in range(NSK):
                ex = expp.tile([P, 2, 512], BF16, tag="ex", name="ex")
                if sk in GP_SKS:
                    # exp(x) ~= (1 + x/16)^16 split across DVE + gpsimd
                    ta = gxp.tile([P, 2, 512], F32, tag="ga")
                    tb = gxp.tile([P, 2, 512], F32, tag="gb")
                    nc.vector.tensor_scalar(
                        ta, sc_prev, 1.0 / (16.0 * TEMP), 1.0,
                        ALU.mult, ALU.add)
                    nc.gpsimd.tensor_mul(tb, ta, ta)      # ^2
                    nc.gpsimd.tensor_mul(ta, tb, tb)      # ^4
                    nc.gpsimd.tensor_mul(tb, ta, ta)      # ^8
                    nc.gpsimd.tensor_mul(ex, tb, tb)      # ^16 -> bf16
                else:
                    nc.scalar.activation(ex, sc_prev, AF.Exp,
                                         scale=1.0 / TEMP)
                if sk + 1 < NSK:
                    sc_prev = emit_scores(sk + 1)
                if sk in pump_sks and work:
                    work.pop(0)()
                for hh in range(2):
                    nc.tensor.matmul(
                        ot_ps[hh],
                        lhsT=v_sb[:, sk, 2 * c + hh, :],
                        rhs=ex[:, hh, :],
                        start=(sk == 0),
                        stop=(sk == NSK - 1),
                    )
            # epilogue: normalize rows 1:65 by denom row 0, no PE
            # involvement (custom DVE ops require partition-0 APs)
            for hh in range(2):
                dn = rcp.tile([1, 512], F32, tag="dn")
                nc.vector.tensor_copy(dn, ot_ps[hh][0:1, :])
                rc = rcp.tile([1, 512], F32, tag="rc")
                nc.vector.reciprocal_approx_fast(rc, dn)
                bc = bcp.tile([65, 512], F32, tag="bc")
                nc.gpsimd.partition_broadcast(bc, rc)
                oo = oop.tile([65, 512], BF16, tag="oo")
                nc.vector.tensor_mul(oo, ot_ps[hh], bc)
                nc.sync.dma_start(
                    ot_sb[hh * 64:(hh + 1) * 64, c, sq * 512:(sq + 1) * 512],
                    oo[1:65, :],
                )

        with tc.tile_pool(name="wkq", bufs=1) as wkq:
            wk = wkq.tile([P, KD, F], BF16)
            xk = wkq.tile([P, KD, SK], BF16)
            wq = wkq.tile([P, KD, F], BF16)
            xq = wkq.tile([P, KD, SQ], BF16)

            with tc.tile_pool(name="vload", bufs=1) as vload:
                wv = vload.tile([P, KD, F], BF16)
                xv = vload.tile([P, KD, SK], BF16)
                # all input DMAs up front, priority order; wv/xv split
                # per-kd chunk so the first V-proj matmuls start early
                wv_r = ins["wvT"].rearrange("(c p) f -> p c f", p=P)
                for kd in range(KD):
                    nc.sync.dma_start(wv[:, kd, :], wv_r[:, kd, :])
                    nc.sync.dma_start(xv[:, kd, :], xv_r[:, kd, :])
                nc.sync.dma_start(wk, ins["wkT"].rearrange("(c p) f -> p c f", p=P))
                nc.sync.dma_start(xk, xk_r)
                nc.sync.dma_start(wq, ins["wqT"].rearrange("(c p) f -> p c f", p=P))
                nc.sync.dma_start(xq, xq_r)

                with tc.tile_pool(name="projps", bufs=2, space="PSUM") as pps:
                    # ------- V projection: V_all[Sk, F] (SBUF-resident) -----
                    for n in range(2):
                        for sk in range(NSK):
                            ps = pps.tile([P, 512], F32, tag="ps", name="vp")
                            for kd in range(KD):
                                nc.tensor.matmul(
                                    ps,
                                    lhsT=xv[:, kd, sk * P:(sk + 1) * P],
                                    rhs=wv[:, kd, n * 512:(n + 1) * 512],
                                    start=(kd == 0),
                                    stop=(kd == KD - 1),
                                )
                            nc.vector.tensor_copy(
                                v_sb[:, sk, n * 8:(n + 1) * 8, 1:65],
                                ps.rearrange("p (h e) -> p h e", h=8),
                            )

            with tc.tile_pool(name="projps2", bufs=2, space="PSUM") as pps:

                def k_group(c, n):
                    def emit():
                        if c not in kt_tiles:
                            kt_tiles[c] = ktp.tile([P, SK], BF16, tag="kt", name="ktc")
                        ktc = kt_tiles[c]
                        ps = pps.tile([P, 512], F32, tag="ps", name="kp")
                        for kd in range(KD):
                            nc.tensor.matmul(
                                ps,
                                lhsT=wk[:, kd, c * P:(c + 1) * P],
                                rhs=xk[:, kd, n * 512:(n + 1) * 512],
                                start=(kd == 0),
                                stop=(kd == KD - 1),
                            )
                        nc.vector.tensor_copy(
                            ktc[:, n * 512:(n + 1) * 512], ps)
                    return emit

                def q_group(c, n):
                    def emit():
                        if c not in qt_tiles:
                            qt_tiles[c] = qtp.tile([P, SQ], BF16, tag="qt", name="qtc")
                        qtc = qt_tiles[c]
                        ps = pps.tile([P, 512], F32, tag="ps", name="qp")
                        for kd in range(KD):
                            nc.tensor.matmul(
                                ps,
                                lhsT=wq[:, kd, c * P:(c + 1) * P],
                                rhs=xq[:, kd, n * 512:(n + 1) * 512],
                                start=(kd == 0),
                                stop=(kd == KD - 1),
                            )
                        nc.vector.tensor_copy(
                            qtc[:, n * 512:(n + 1) * 512], ps)
                    return emit

                with tc.tile_pool(name="gxp", bufs=2) as gxp:
                    # chunk 0 projections up front
                    for n in range(SK // 512):
                        k_group(0, n)()
                    for n in range(SQ // 512):
                        q_group(0, n)()
                    # main loop: attention on c, proj groups for c+1 pumped in
                    for c in range(NF - 1):
                        work = [k_group(c + 1, n) for n in range(SK // 512)]
                        work += [q_group(c + 1, n) for n in range(SQ // 512)]
                        attn(c, 0, work)
                        attn(c, 1, work)
                        for w in work:   # leftovers (shouldn't happen)
                            w()

        # ---------------- final: out proj + residual + layernorm -----------
        with (
            tc.tile_pool(name="wpp", bufs=1) as wpp,
            tc.tile_pool(name="lnc", bufs=1) as lnc,
            tc.tile_pool(name="qres", bufs=8) as qrp,
            tc.tile_pool(name="lnx", bufs=9) as lnx,
            tc.tile_pool(name="lnxn", bufs=3) as lnxn,
            tc.tile_pool(name="stat", bufs=32) as stp,
            tc.tile_pool(name="fps", bufs=2, space="PSUM") as fps,
            tc.tile_pool(name="gxp2", bufs=2) as gxp,
        ):
            wp = wpp.tile([P, NF, D], BF16)
            nc.sync.dma_start(wp, ins["wpT"].rearrange("(c p) f -> p c f", p=P))
            scale_sb = lnc.tile([P, 2, 512], F32)
            nc.sync.dma_start(
                scale_sb, ins["scale_b"].rearrange("p (a b) -> p a b", a=2))
            offset_sb = lnc.tile([P, 2, 512], F32)
            nc.sync.dma_start(
                offset_sb, ins["offset_b"].rearrange("p (a b) -> p a b", a=2))

            parts = {}

            def final_mm(qc):
                def emit():
                    qr = qrp.tile([P, 2, 512], BF16, tag="qr")
                    nc.sync.dma_start(
                        qr,
                        ins["qres"][qc * P:(qc + 1) * P, :].rearrange(
                            "p (a b) -> p a b", a=2),
                    )
                    x = lnx.tile([P, 2, 512], F32, tag="x", name="x")
                    for d in range(2):
                        fp = fps.tile([P, 512], F32, tag="fp")
                        for f in range(NF):
                            nc.tensor.matmul(
                                fp,
                                lhsT=ot_sb[:, f, qc * P:(qc + 1) * P],
                                rhs=wp[:, f, d * 512:(d + 1) * 512],
                                start=(f == 0),
                                stop=(f == NF - 1),
                            )
                        nc.vector.tensor_add(x[:, d, :], fp, qr[:, d, :])
                    stats = stp.tile([P, 2, 6], F32, tag="st", name="st")
                    for gsub in range(2):
                        nc.vector.bn_stats(stats[:, gsub, :], x[:, gsub, :])
                    mv = stp.tile([P, 2], F32, tag="mv", name="mv")
                    nc.vector.bn_aggr(mv, stats)
                    parts[qc] = (x, mv)
                return emit

            def final_ln(qc):
                # rstd = rsqrt(var*D/(D-1)) via DVE-only Newton (seed 1/v):
                # avoids the ACT Sqrt table-set thrash against Exp. EPS=1e-9
                # vanishes in fp32 rounding for std ~ O(1) (reference rounds
                # identically). var~1 here so 3 iterations reach ~1e-6 rel.
                x, mv = parts[qc]
                v = stp.tile([P, 1], F32, tag="v", name="v")
                nc.vector.tensor_scalar_mul(v, mv[:, 1:2],
                                            float(D) / float(D - 1))
                y = stp.tile([P, 1], F32, tag="y", name="y")
                nc.vector.reciprocal(y, v)
                for _ in range(3):
                    t = stp.tile([P, 1], F32, tag="t", name="t")
                    nc.vector.tensor_mul(t, y, y)
                    nc.vector.tensor_mul(t, t, v)
                    nc.vector.tensor_scalar(t, t, -0.5, 1.5,
                                            ALU.mult, ALU.add)
                    nc.vector.tensor_mul(y, y, t)
                xn = lnxn.tile([P, 2, 512], F32, tag="xn", name="xn")
                nc.vector.tensor_scalar(xn, x, mv[:, 0:1], y,
                                        ALU.subtract, ALU.mult)
                nc.vector.tensor_mul(xn, xn, scale_sb)
                nc.vector.tensor_add(xn, xn, offset_sb)
                nc.sync.dma_start(
                    out_ap[qc * P:(qc + 1) * P, :],
                    xn.rearrange("p a b -> p (a b)"),
                )

            # last head-pair chunk: final-proj matmuls+stats pumped into the
            # second half's sk stream; LN tails run after the last exp
            c = NF - 1
            attn(c, 0, [])
            # only sq0's q-chunks may be pumped here: sq1's ot_sb columns
            # are written by this very attention call's epilogue
            work = [final_mm(qc) for qc in range(4)]
            attn(c, 1, work, pump_sks=PUMP_SKS_LAST)
            for w in work:
                w()
            for qc in range(4, SQ // P):
                final_mm(qc)()
            for qc in range(SQ // P):
                final_ln(qc)


def build_program():
    nc = bacc.Bacc("TRN2", debug=False, target_bir_lowering=False)
    shapes = {
        "qT": ([D, SQ], BF16), "kT": ([D, SK], BF16), "vT": ([D, SK], BF16),
        "qres": ([SQ, D], BF16),
        "wqT": ([D, F], BF16), "wkT": ([D, F], BF16), "wvT": ([D, F], BF16),
        "wpT": ([F, D], BF16),
        "scale_b": ([P, D], F32), "offset_b": ([P, D], F32),
    }
    ins = {k: nc.dram_tensor(k, shp, dt, kind="ExternalInput").ap()
           for k, (shp, dt) in shapes.items()}
    out = nc.dram_tensor("out", [SQ, D], F32, kind="ExternalOutput").ap()
    with tile.TileContext(nc) as tc:
        _mha_kernel(tc, out, ins)
    nc.compile()
    return nc


_PROGRAM = None


def _get_program():
    global _PROGRAM
    if _PROGRAM is None:
        _PROGRAM = build_program()
    return _PROGRAM


def make_in_maps(q, k, v, Wq, Wk, Wv, Wp, scale, offset):
    import ml_dtypes
    f = np.float32
    bf = ml_dtypes.bfloat16
    q = np.asarray(q, f)
    k16 = np.asarray(k, f).astype(bf)
    v16 = np.asarray(v, f).astype(bf)
    q16 = q.astype(bf)
    wqT = np.ascontiguousarray(
        np.asarray(Wq, f).transpose(2, 0, 1).reshape(D, F).astype(bf))
    wkT = np.ascontiguousarray(
        np.asarray(Wk, f).transpose(2, 0, 1).reshape(D, F).astype(bf))
    wvT = np.ascontiguousarray(
        np.asarray(Wv, f).transpose(2, 0, 1).reshape(D, F).astype(bf))
    wpT = np.ascontiguousarray(np.asarray(Wp, f).T.astype(bf))
    scale_b = np.ascontiguousarray(
        np.broadcast_to(np.asarray(scale, f), (P, D)))
    offset_b = np.ascontiguousarray(
        np.broadcast_to(np.asarray(offset, f), (P, D)))
    in_maps = []
    for c in range(N_CORES):
        b, half = divmod(c, 2)
        sl = slice(half * SQ, (half + 1) * SQ)
        in_maps.append({
            "qT": np.ascontiguousarray(q16[b, sl].T),
            "qres": np.ascontiguousarray(q16[b, sl]),
            "kT": np.ascontiguousarray(k16[b].T),
            "vT": np.ascontiguousarray(v16[b].T),
            "wqT": wqT, "wkT": wkT, "wvT": wvT, "wpT": wpT,
            "scale_b": scale_b, "offset_b": offset_b,
        })
    return in_maps


def kernel(q, k, v, Wq, Wk, Wv, Wp, scale, offset):
    global LAST_RESULT
    in_maps = make_in_maps(q, k, v, Wq, Wk, Wv, Wp, scale, offset)
    nc = _get_program()
    res = run_bass_kernel_spmd(nc, in_maps, list(range(N_CORES)))
    LAST_RESULT = res
    out = np.empty((B, S, D), np.float32)
    for c in range(N_CORES):
        b, half = divmod(c, 2)
        out[b, half * SQ:(half + 1) * SQ] = res.results[c]["out"]
    return out
